# revision 1
# baseline (speedup 1.0000x reference)
"""Trainium2 Bass kernel for nn_DisCA (dual conv-block + channel attention).

Data-parallel over batch: 8 batch items -> 8 NeuronCores, one image per core.
Conv weights / BN affine replicated. BatchNorm batch statistics are obtained
with a single cross-core AllReduce of per-channel (sum, sumsq) for both conv
blocks (a [4,512] f32 tensor). The BN affine transform is never applied to the
big feature maps: it is folded algebraically into the attention-score matrix,
    scores = a1[c]*a2[d]*S[c,d] + (a1*r1)[c]*b2[d] + b1[c]*(a2*r2 + N*b2)[d]
where S is the raw (pre-BN) Gram matrix and r_i are local per-channel row sums,
so the raw score matmul overlaps the AllReduce latency.

Layouts (per core):
  x/x1/x2: [128, 4096]  - channel-chunk k (4 chunks of 128ch) at cols 1024k,
                          spatial n=H*W=1024 flattened row-major.
  w1t:     [128, 1024]  - W1^T [512,256] as 4 K-chunks of [128,256].
  w2t:     [128, 9216]  - per tap t=kh*3+kw, K-chunk k: W2[:,:,kh,kw]^T
                          [256,512] chunk [128,512] at cols (2t+k)*512.
  conv1 out Y1 is stored zero-padded spatially ([34x34] per channel) so the
  3x3 conv is 9 shifted 1x1 matmuls with no edge fixups.
  conv2 output F is produced TRANSPOSED ([n, c] = 8 chunks of [128 n, 512 c])
  so the score matmuls need no explicit transposes.
"""

import os
import sys

for _p in ("/opt/trn_rl_repo", "/root/.axon_site/_ro/trn_rl_repo"):
    if os.path.isdir(_p) and _p not in sys.path:
        sys.path.insert(0, _p)

import numpy as np

import concourse.bacc as bacc
import concourse.mybir as mybir
from concourse.tile import TileContext, add_dep_helper
from concourse.bass_utils import run_bass_kernel_spmd
from concourse.masks import make_identity

F32 = mybir.dt.float32
F32R = mybir.dt.float32r


def _r(ap):
    """Reinterpret an fp32 AP as float32r (single-pass full-rate PE mode)."""
    return ap.bitcast(F32R)
AF = mybir.ActivationFunctionType
ALU = mybir.AluOpType

NCORES = 8
B, C, H, W = 8, 512, 32, 32
N = H * W                      # 1024 spatial positions per image
CMID = 256                     # conv1 output channels
HP = H + 2                     # padded spatial dim
NPAD = HP * HP                 # 1156
BN_EPS = 1e-5
LRELU_SLOPE = 0.01
M_TOTAL = float(B * N)         # BN statistic count (full batch)

KC = C // 128                  # 4 channel chunks of x
KM = CMID // 128               # 2 channel chunks of mid features


def build_kernel(stage=99):
    nc = bacc.Bacc("TRN2", target_bir_lowering=False, debug=False,
                   num_devices=NCORES)

    # ---- DRAM I/O -------------------------------------------------------
    x1d = nc.dram_tensor("x1s", [128, 4096], F32, kind="ExternalInput")
    x2d = nc.dram_tensor("x2s", [128, 4096], F32, kind="ExternalInput")
    xd = nc.dram_tensor("xs", [128, 4096], F32, kind="ExternalInput")
    w1d = nc.dram_tensor("w1t", [128, 1024], F32, kind="ExternalInput")
    w2d = nc.dram_tensor("w2t", [128, 9216], F32, kind="ExternalInput")
    # vecs rows: 0=b2, 1=gamma, 2=bn_bias, 3=beta(col0), 4=b1(cols 0:256)
    vecd = nc.dram_tensor("vecs", [8, 512], F32, kind="ExternalInput")
    outd = nc.dram_tensor("out", [128, 4096], F32, kind="ExternalOutput")

    cc_in = nc.dram_tensor("cc_in", [1, 2048], F32, kind="Internal")
    cc_out = nc.dram_tensor("cc_out", [1, 2048], F32, kind="Internal",
                            addr_space="Shared")
    cw_in = nc.dram_tensor("cw_in", [1, 8], F32, kind="Internal")
    cw_out = nc.dram_tensor("cw_out", [1, 8], F32, kind="Internal",
                            addr_space="Shared")

    with TileContext(nc, num_cores=NCORES) as tc:
        with (
            tc.tile_pool(name="const", bufs=1) as const,
            tc.tile_pool(name="big", bufs=1) as big,
            tc.tile_pool(name="work", bufs=2) as work,
            tc.tile_pool(name="vec", bufs=1) as vec,
            tc.tile_pool(name="ps", bufs=8, space="PSUM") as ps,
        ):
            # ---- constants / small tiles -------------------------------
            identity = const.tile([128, 128], F32)
            make_identity(nc, identity)
            ones_col = const.tile([128, 1], F32)   # lhsT for partition sums
            nc.vector.memset(ones_col[:], 1.0)
            nc.scalar.copy(_r(ones_col[:]), ones_col[:])
            ones_row = const.tile([1, 128], F32)   # K=1 lhsT for broadcasts
            nc.vector.memset(ones_row[:], 1.0)
            ones_row_r = const.tile([1, 128], F32)
            nc.vector.memset(ones_row_r[:], 1.0)
            nc.scalar.copy(_r(ones_row_r[:]), ones_row_r[:])

            # tiny warmup AllReduce: pays the collective setup cost while
            # the input DMAs stream, so the real stats AR hits a warm path
            warm = const.tile([1, 8], F32)
            nc.vector.memset(warm[:], 1.0)
            nc.sync.dma_start(out=cw_in[:], in_=warm[:])
            nc.gpsimd.collective_compute(
                "AllReduce", ALU.add,
                replica_groups=[list(range(NCORES))],
                ins=[cw_in[:]], outs=[cw_out[:]])

            b2row = const.tile([1, 512], F32)
            nc.sync.dma_start(out=_r(b2row[:]), in_=_r(vecd[0:1, :]))
            b1pp = const.tile([128, KM], F32)      # b1 per-partition chunks
            for m in range(KM):
                nc.sync.dma_start(out=b1pp[:, m:m + 1],
                                  in_=vecd[4:5, 128 * m:128 * (m + 1)])

            # ---- big persistent tiles + input DMAs ---------------------
            w1t = big.tile([128, 1024], F32)
            nc.sync.dma_start(out=_r(w1t[:]), in_=_r(w1d[:]))
            x1s = work.tile([128, 4096], F32, tag="xin")
            for j in range(4):
                nc.sync.dma_start(out=_r(x1s[:, 1024 * j:1024 * (j + 1)]),
                                  in_=_r(x1d[:, 1024 * j:1024 * (j + 1)]))
            w2t = big.tile([128, 9216], F32)
            for j in range(6):
                nc.sync.dma_start(out=_r(w2t[:, 1536 * j:1536 * (j + 1)]),
                                  in_=_r(w2d[:, 1536 * j:1536 * (j + 1)]))
            x2s = work.tile([128, 4096], F32, tag="xin")
            for j in range(4):
                nc.sync.dma_start(out=_r(x2s[:, 1024 * j:1024 * (j + 1)]),
                                  in_=_r(x2d[:, 1024 * j:1024 * (j + 1)]))

            gb = const.tile([1, 1024], F32)        # gamma | bn_bias
            nc.sync.dma_start(out=gb[0:1, 0:512], in_=vecd[1:2, :])
            nc.sync.dma_start(out=gb[0:1, 512:1024], in_=vecd[2:3, :])
            betar = const.tile([1, 1], F32)
            nc.sync.dma_start(out=betar[:], in_=vecd[3:4, 0:1])

            # conv1 output, 3 horizontally-pre-shifted copies (kw = 0,1,2),
            # each vertically zero-padded to 34 rows of 32 contiguous cols:
            #   y1c[kw][k][c, r, w] = Y1[c_chunk k][r-1, w + kw - 1]
            # so conv2's stationary operand (s, kh, kw) is the contiguous
            # 128-elem slice at rows 4s+kh .. 4s+kh+3 of copy kw.
            NROW = HP * W                       # 1088 elems per copy/chunk
            y1c = big.tile([128, 3 * KM * NROW], F32)
            nc.vector.memset(y1c[:], 0.0)
            nc.scalar.copy(_r(y1c[:]), y1c[:])

            def y1base(kw, k):
                return (kw * KM + k) * NROW
            f1t = big.tile([128, 4096], F32)
            f2t = big.tile([128, 4096], F32)
            stats = const.tile([1, 2048], F32)     # local r1|s1|r2|s2
            ar = const.tile([1, 2048], F32)        # all-reduced stats

            def bail(src512):
                for m in range(8):
                    nc.sync.dma_start(
                        out=outd[:, 512 * m:512 * (m + 1)], in_=src512)

            # ---- one conv block: x -> conv1 -> pad -> conv2 -> lrelu ---
            def conv_block(xin, ft, si, upto=None):
                # conv1: Y1[cmid, n] = W1 @ x, written into shifted copies
                for m in range(KM):
                    for n2 in range(2):
                        acc = ps.tile([128, 512], F32, tag="ps")
                        for k in range(KC):
                            nc.tensor.matmul(
                                acc[:],
                                _r(w1t[:, 256 * k + 128 * m:256 * k + 128 * (m + 1)]),
                                _r(xin[:, 1024 * k + 512 * n2:1024 * k + 512 * (n2 + 1)]),
                                start=(k == 0), stop=(k == KC - 1))
                        # psum holds rows 16*n2 .. 16*n2+15 (32 cols each)
                        accv = acc[:].rearrange("p (r c) -> p r c", c=W)
                        row0 = (1 + 16 * n2) * W
                        # center copy (kw=1): straight contiguous store
                        nc.scalar.activation(
                            _r(y1c[:, y1base(1, m) + row0:y1base(1, m) + row0 + 512]),
                            acc[:], AF.Identity, bias=b1pp[:, m:m + 1])
                        # kw=0: shift right one col (src cols 0..30 -> 1..31)
                        d0 = y1c[:, y1base(0, m):y1base(0, m) + NROW].rearrange(
                            "p (r c) -> p r c", c=W)
                        nc.scalar.activation(
                            _r(d0[:, 1 + 16 * n2:17 + 16 * n2, 1:32]),
                            accv[:, :, 0:31], AF.Identity,
                            bias=b1pp[:, m:m + 1])
                        # kw=2: shift left one col (src cols 1..31 -> 0..30)
                        d2 = y1c[:, y1base(2, m):y1base(2, m) + NROW].rearrange(
                            "p (r c) -> p r c", c=W)
                        nc.scalar.activation(
                            _r(d2[:, 1 + 16 * n2:17 + 16 * n2, 0:31]),
                            accv[:, :, 1:32], AF.Identity,
                            bias=b1pp[:, m:m + 1])

                if upto == "conv1":
                    return
                # conv2 (3x3) -> transposed output F^T[n, c], 8 spatial chunks
                accs = []
                for s in range(8):
                    acc = ps.tile([128, 512], F32, tag="ps")
                    accs.append(acc)
                    # bias: + b2[c] on every row (rank-1, K=1)
                    nc.tensor.matmul(acc[:], _r(ones_row_r[:]), _r(b2row[:]),
                                     start=True, stop=False)
                for kh in range(3):
                    for kw in range(3):
                        t = kh * 3 + kw
                        for k in range(KM):
                            rhs = w2t[:, (2 * t + k) * 512:(2 * t + k + 1) * 512]
                            last = (kh == 2 and kw == 2 and k == KM - 1)
                            for s in range(8):
                                off = y1base(kw, k) + (4 * s + kh) * W
                                nc.tensor.matmul(accs[s][:],
                                                 _r(y1c[:, off:off + 128]), _r(rhs),
                                                 start=False, stop=last)
                for s in range(8):
                    nc.scalar.activation(_r(ft[:, 512 * s:512 * (s + 1)]),
                                         accs[s][:], AF.Lrelu,
                                         alpha=LRELU_SLOPE)

                if upto == "conv2":
                    return
                # per-channel raw sums r and sumsq s (free layout [1,512])
                racc = ps.tile([1, 512], F32, tag="ps")
                for k in range(8):
                    nc.tensor.matmul(racc[:], _r(ones_col[:]),
                                     _r(ft[:, 512 * k:512 * (k + 1)]),
                                     start=(k == 0), stop=(k == 7))
                nc.scalar.copy(stats[0:1, 1024 * si:1024 * si + 512], racc[:])
                qacc = ps.tile([1, 512], F32, tag="ps")
                for k in range(8):
                    sq = work.tile([128, 512], F32, tag="sq")
                    nc.scalar.activation(_r(sq[:]), ft[:, 512 * k:512 * (k + 1)],
                                         AF.Square)
                    nc.tensor.matmul(qacc[:], _r(ones_col[:]), _r(sq[:]),
                                     start=(k == 0), stop=(k == 7))
                nc.scalar.copy(stats[0:1, 1024 * si + 512:1024 * (si + 1)],
                               qacc[:])

            def main():
                if stage <= 3:
                    conv_block(x1s, f1t, 0,
                               upto={1: "conv1", 2: "conv2"}.get(stage))
                    bail(y1c[:, 0:512] if stage == 1 else f1t[:, 0:512])
                    return
                conv_block(x1s, f1t, 0)
                # block-1 stats AllReduce issued now: its latency hides
                # under conv_block(x2)'s ~60us of compute
                nc.sync.dma_start(out=cc_in[0:1, 0:1024],
                                  in_=stats[0:1, 0:1024])
                nc.gpsimd.collective_compute(
                    "AllReduce", ALU.add,
                    replica_groups=[list(range(NCORES))],
                    ins=[cc_in[0:1, 0:1024]], outs=[cc_out[0:1, 0:1024]])
                # x reuses x1's slot (x1 is dead after its conv1)
                xs = work.tile([128, 4096], F32, tag="xin")
                for j in range(4):
                    nc.sync.dma_start(out=_r(xs[:, 1024 * j:1024 * (j + 1)]),
                                      in_=_r(xd[:, 1024 * j:1024 * (j + 1)]))
                conv_block(x2s, f2t, 1)
                if stage == 4:
                    bail(f2t[:, 0:512])
                    return
                tail(xs)

            def tail(xs):
                # ---- block-2 BN statistics AllReduce (block 1 already
                # in flight since the end of conv_block(x1)) ---------------
                nc.sync.dma_start(out=cc_in[0:1, 1024:2048],
                                  in_=stats[0:1, 1024:2048])
                nc.gpsimd.collective_compute(
                    "AllReduce", ALU.add,
                    replica_groups=[list(range(NCORES))],
                    ins=[cc_in[0:1, 1024:2048]], outs=[cc_out[0:1, 1024:2048]])
                nc.sync.dma_start(out=ar[:], in_=cc_out[:])

                # ---- BN affine params (free layout, all on partition 0) ----
                # a_i = gamma * rsqrt(var+eps); b_i = bn_bias - mean*a_i
                # Both conv blocks at once via strided [2,512] free APs to
                # minimize cross-engine hops on the post-AR critical path.
                gam = gb[0:1, 0:512]
                bnb = gb[0:1, 512:1024]
                # aball slices: a1 | b1bn | a2 | b2bn
                aball = vec.tile([1, 2048], F32)
                tmp = vec.tile([1, 2048], F32)
                r_both = ar[0:1, :].rearrange("o (i c) -> o i c", c=512)[:, 0::2, :]
                s_both = ar[0:1, :].rearrange("o (i c) -> o i c", c=512)[:, 1::2, :]
                mean2 = tmp[0:1, 0:1024].rearrange("o (i c) -> o i c", c=512)
                var2 = tmp[0:1, 1024:2048].rearrange("o (i c) -> o i c", c=512)
                a_both = aball[0:1, :].rearrange("o (i c) -> o i c", c=512)[:, 0::2, :]
                b_both = aball[0:1, :].rearrange("o (i c) -> o i c", c=512)[:, 1::2, :]
                nc.vector.tensor_scalar_mul(mean2, r_both, 1.0 / M_TOTAL)
                nc.vector.tensor_scalar(var2, s_both, 1.0 / M_TOTAL, BN_EPS,
                                        op0=ALU.mult, op1=ALU.add)
                mm2 = vec.tile([1, 1024], F32)
                mm2v = mm2[0:1, :].rearrange("o (i c) -> o i c", c=512)
                nc.vector.tensor_mul(mm2v, mean2, mean2)
                nc.vector.tensor_sub(var2, var2, mm2v)
                nc.scalar.activation(mm2v, var2, AF.Sqrt)
                nc.vector.reciprocal(var2, mm2v)
                for i in range(2):
                    a_i = aball[0:1, 1024 * i:1024 * i + 512]
                    b_i = aball[0:1, 1024 * i + 512:1024 * (i + 1)]
                    v_i = tmp[0:1, 1024 + 512 * i:1024 + 512 * (i + 1)]
                    m_i = tmp[0:1, 512 * i:512 * (i + 1)]
                    nc.vector.tensor_mul(a_i, v_i, gam)
                    nc.vector.tensor_mul(v_i, m_i, a_i)
                    nc.vector.tensor_sub(b_i, bnb, v_i)

                # rank-1 score correction vectors: u=a1*r1, w=a2*r2+N*b2bn
                a1_r = aball[0:1, 0:512]
                b1bn_r = aball[0:1, 512:1024]
                a2_r = aball[0:1, 1024:1536]
                b2bn_r = aball[0:1, 1536:2048]
                uw = vec.tile([1, 1024], F32)   # u | w
                u_r = uw[0:1, 0:512]
                w_r = uw[0:1, 512:1024]
                nc.vector.tensor_mul(u_r, a1_r, stats[0:1, 0:512])
                nc.vector.tensor_mul(w_r, a2_r, stats[0:1, 1024:1536])
                nc.vector.scalar_tensor_tensor(
                    w_r, b2bn_r, float(N), w_r, op0=ALU.mult, op1=ALU.add)

                # a1-side per-partition [128, 4] quantities via
                # partition-scatter loads (dest[p, j] = src[128j + p]);
                # local r1 comes from cc_in (pre-AR), global stats from cc_out
                r1pp = vec.tile([128, KC], F32)
                s1pp = vec.tile([128, KC], F32)
                gpp = vec.tile([128, KC], F32)
                nc.sync.dma_start(
                    out=gpp[:],
                    in_=vecd[1:2, :].rearrange("o (j p) -> o p j", p=128))
                nc.sync.dma_start(
                    out=r1pp[:],
                    in_=cc_out[0:1, 0:512].rearrange("o (j p) -> o p j", p=128))
                nc.sync.dma_start(
                    out=s1pp[:],
                    in_=cc_out[0:1, 512:1024].rearrange("o (j p) -> o p j", p=128))
                a1pp = vec.tile([128, KC], F32)
                nc.vector.tensor_scalar_mul(r1pp[:], r1pp[:], 1.0 / M_TOTAL)
                nc.vector.tensor_mul(a1pp[:], r1pp[:], r1pp[:])    # mean^2
                nc.vector.tensor_scalar(s1pp[:], s1pp[:], 1.0 / M_TOTAL,
                                        BN_EPS, op0=ALU.mult, op1=ALU.add)
                nc.vector.tensor_sub(s1pp[:], s1pp[:], a1pp[:])    # var+eps
                nc.scalar.activation(s1pp[:], s1pp[:], AF.Sqrt)
                nc.vector.reciprocal(s1pp[:], s1pp[:])
                nc.vector.tensor_mul(a1pp[:], gpp[:], s1pp[:])

                # a2 broadcast tile [128, 512] (outer product with ones)
                bc = ps.tile([128, 512], F32, tag="ps")
                nc.tensor.matmul(bc[:], ones_row[:], a2_r,
                                 start=True, stop=True)
                a2b = const.tile([128, 512], F32)
                nc.scalar.copy(a2b[:], bc[:])
                # beta per-partition [128,1]
                bps = ps.tile([128, 1], F32, tag="ps")
                nc.tensor.matmul(bps[:], ones_row[:], betar[:],
                                 start=True, stop=True)
                betapp = vec.tile([128, 1], F32)
                nc.scalar.copy(betapp[:], bps[:])
                if stage == 5:
                    bail(a2b[:])
                    return

                # ---- scores + softmax-exp + transpose + apply ----------
                # Raw Gram matmuls first (no AR dependency) so the PE works
                # while the AllReduce completes; everything AR-dependent
                # follows, pipelined per c-chunk m:
                #   rank1(m) -> corrections(m) -> exp(m) -> transpose(m) ->
                #   final(m) while (m+1) runs its DVE/ACT stages.
                et = y1c                             # E^T overlaid on dead y1c
                scvec = vec.tile([128, KC], F32)     # beta/sumexp per c-chunk
                ssb = big.tile([128, 2048], F32)   # raw Gram in SBUF (fp32)
                for m in range(KC):
                    sacc = ps.tile([128, 512], F32, tag="ps")
                    for k in range(8):
                        nc.tensor.matmul(
                            sacc[:],
                            _r(f1t[:, 512 * k + 128 * m:512 * k + 128 * (m + 1)]),
                            _r(f2t[:, 512 * k:512 * (k + 1)]),
                            start=(k == 0), stop=(k == 7))
                    nc.scalar.copy(ssb[:, 512 * m:512 * (m + 1)], sacc[:])

                # scheduler fence: nothing below may be reordered above this
                # point, so the AR-dependent matmuls cannot hoist ahead of
                # the Gram matmuls and stall the in-order PE queue
                tc.no_sync_barrier()

                # a2 broadcast tile [128, 512] (outer product with ones)
                bc = ps.tile([128, 512], F32, tag="ps")
                nc.tensor.matmul(bc[:], ones_row[:], a2_r,
                                 start=True, stop=True)
                a2b = const.tile([128, 512], F32)
                nc.scalar.copy(a2b[:], bc[:])
                # beta per-partition [128,1]
                bps = ps.tile([128, 1], F32, tag="ps")
                nc.tensor.matmul(bps[:], ones_row[:], betar[:],
                                 start=True, stop=True)
                betapp = vec.tile([128, 1], F32)
                nc.scalar.copy(betapp[:], bps[:])
                if stage == 5:
                    bail(a2b[:])
                    return

                r1s = []
                for m in range(KC):
                    r1acc = ps.tile([128, 512], F32, tag="ps")
                    r1s.append(r1acc)
                    nc.tensor.matmul(r1acc[:], u_r[:, 128 * m:128 * (m + 1)],
                                     b2bn_r, start=True, stop=False)
                    nc.tensor.matmul(r1acc[:], b1bn_r[:, 128 * m:128 * (m + 1)],
                                     w_r, start=False, stop=True)
                for m in range(KC):
                    # scores = (S * a2[d]) * a1[c] + rank1
                    tmul = work.tile([128, 512], F32, tag="tmul")
                    nc.vector.tensor_mul(tmul[:], ssb[:, 512 * m:512 * (m + 1)],
                                         a2b[:])
                    sc = work.tile([128, 512], F32, tag="scores")
                    nc.vector.scalar_tensor_tensor(
                        sc[:], tmul[:], a1pp[:, m:m + 1], r1s[m][:],
                        op0=ALU.mult, op1=ALU.add)
                    # E = exp(scores - rowmax), sumexp accumulated for free
                    nmx = vec.tile([128, 1], F32, tag="nmx")
                    nc.vector.tensor_reduce(nmx[:], sc[:],
                                            axis=mybir.AxisListType.X,
                                            op=ALU.max, negate=True)
                    esum = vec.tile([128, 1], F32, tag="esum")
                    ee = work.tile([128, 512], F32, tag="ee")
                    nc.scalar.activation(ee[:], sc[:], AF.Exp, bias=nmx[:],
                                         accum_out=esum[:])
                    nc.vector.reciprocal(esum[:], esum[:])
                    nc.vector.tensor_mul(scvec[:, m:m + 1], esum[:], betapp[:])
                    # transpose E chunk into et
                    for j in range(KC):
                        tp = ps.tile([128, 128], F32, tag="ps")
                        nc.tensor.transpose(tp[:], ee[:, 128 * j:128 * (j + 1)],
                                            identity[:])
                        nc.scalar.copy(_r(et[:, 512 * j + 128 * m:512 * j + 128 * (m + 1)]),
                                       tp[:])
                    if stage == 6:
                        continue
                    # out[c, n] = (beta/sumexp)[c] * sum_d E^T[d,c] x[d,n]
                    # (c-chunk m only needs its own four transposes)
                    for n2 in range(2):
                        oacc = ps.tile([128, 512], F32, tag="ps")
                        for k in range(KC):
                            nc.tensor.matmul(
                                oacc[:],
                                _r(et[:, 512 * k + 128 * m:512 * k + 128 * (m + 1)]),
                                _r(xs[:, 1024 * k + 512 * n2:1024 * k + 512 * (n2 + 1)]),
                                start=(k == 0), stop=(k == KC - 1))
                        ot = work.tile([128, 512], F32, tag="ot")
                        nc.scalar.mul(ot[:], oacc[:], scvec[:, m:m + 1])
                        nc.sync.dma_start(
                            out=outd[:, 1024 * m + 512 * n2:1024 * m + 512 * (n2 + 1)],
                            in_=ot[:])
                if stage == 6:
                    bail(et[:, 0:512])
                    return

            main()

    nc.compile()
    return nc


_NC_CACHE = []


def _get_nc():
    if not _NC_CACHE:
        _NC_CACHE.append(build_kernel())
    return _NC_CACHE[0]


def _prep_shared(w1, b1, w2, b2, gamma, bn_bias, beta):
    w1m = w1.reshape(CMID, C).astype(np.float32)
    w1t = np.ascontiguousarray(
        w1m.T.reshape(KC, 128, CMID).transpose(1, 0, 2).reshape(128, KC * CMID))
    w2t = np.empty((128, 9216), dtype=np.float32)
    for kh in range(3):
        for kw in range(3):
            t = kh * 3 + kw
            wt = w2[:, :, kh, kw].T  # [256 in, 512 out]
            for k in range(KM):
                w2t[:, (2 * t + k) * 512:(2 * t + k + 1) * 512] = \
                    wt[128 * k:128 * (k + 1), :]
    vecs = np.zeros((8, 512), dtype=np.float32)
    vecs[0] = b2
    vecs[1] = gamma
    vecs[2] = bn_bias
    vecs[3, 0] = np.asarray(beta).reshape(-1)[0]
    vecs[4, :CMID] = b1
    return w1t, w2t, vecs


def _chunk_img(img):
    # [512, 1024] -> [128, 4096] with channel chunk k at cols 1024k
    return np.ascontiguousarray(
        img.reshape(KC, 128, N).transpose(1, 0, 2).reshape(128, KC * N))


def kernel(x, x1, x2, w1, b1, w2, b2, gamma, bn_bias, beta, **run_kw):
    nc = _get_nc()
    w1t, w2t, vecs = _prep_shared(w1, b1, w2, b2, gamma, bn_bias, beta)
    in_maps = []
    for i in range(NCORES):
        in_maps.append({
            "x1s": _chunk_img(np.asarray(x1[i], np.float32).reshape(C, N)),
            "x2s": _chunk_img(np.asarray(x2[i], np.float32).reshape(C, N)),
            "xs": _chunk_img(np.asarray(x[i], np.float32).reshape(C, N)),
            "w1t": w1t, "w2t": w2t, "vecs": vecs,
        })
    res = run_bass_kernel_spmd(nc, in_maps, list(range(NCORES)), **run_kw)
    out = np.empty((B, C, H, W), dtype=np.float32)
    for i in range(NCORES):
        o = res.results[i]["out"]  # [128, 4096]
        out[i] = o.reshape(128, KC, N).transpose(1, 0, 2).reshape(C, H, W)
    if run_kw:
        kernel.last_results = res
    return out



# revision 15
# speedup vs baseline: 1.0087x; 1.0087x over previous
"""Trainium2 Bass kernel for nn_DisCA (dual conv-block + channel attention).

Data-parallel over batch: 8 batch items -> 8 NeuronCores, one image per core.
Conv weights / BN affine replicated. BatchNorm batch statistics via per-block
cross-core AllReduce of per-channel (sum, sumsq); BN affine folded into the
attention-score matrix so the raw Gram matmul overlaps the AllReduce:
    scores = a1[c]*a2[d]*S[c,d] + (a1*r1)[c]*b2bn[d] + b1bn[c]*(a2*r2 + N*b2bn)[d]

Key layout choices (per core):
  x/x1/x2: [128, 4096]  - channel-chunk k (4 chunks of 128ch) at cols 1024k,
                          spatial n=H*W=1024 flattened row-major (x in bf16).
  w1t:     [128, 1024]  - W1^T as 4 K-chunks of [128,256].
  w2t:     [128, 9216]  - per tap t, K-chunk k: W2[:,:,kh,kw]^T chunk at
                          cols (2t+k)*512.
  conv1 out Y1 is stored zero-padded 34x34 per channel chunk, so conv2's
  stationary operand for (s,kh,kw,k) is a strided [128,4,32] window and the
  3x3 conv is 9 shifted 1x1 matmuls with no edge fixups and a single store
  per conv1 psum.
  conv2 output F is produced TRANSPOSED ([n, c]) so score matmuls need no
  explicit transposes; softmax E is bf16, transposed via PE, and the final
  attention@x runs in bf16.
All activation functions used (identity/copy/square/parametric_relu/ln/exp)
live in ONE act table set, so no table switches on the critical path; the
BN rsqrt is computed as exp(-0.5*ln(var+eps)).
"""

import os
import sys

for _p in ("/opt/trn_rl_repo", "/root/.axon_site/_ro/trn_rl_repo"):
    if os.path.isdir(_p) and _p not in sys.path:
        sys.path.insert(0, _p)

import numpy as np

import concourse.bacc as bacc
import concourse.mybir as mybir
from concourse.tile import TileContext, add_dep_helper
from concourse.bass_utils import run_bass_kernel_spmd
from concourse.masks import make_identity

F32 = mybir.dt.float32
F32R = mybir.dt.float32r
BF16 = mybir.dt.bfloat16
AF = mybir.ActivationFunctionType
ALU = mybir.AluOpType

NCORES = 8
B, C, H, W = 8, 512, 32, 32
N = H * W                      # 1024 spatial positions per image
CMID = 256                     # conv1 output channels
HP = H + 2                     # padded rows
NROW = HP * W                  # 1088 elems per shifted copy per chunk
BN_EPS = 1e-5
LRELU_SLOPE = 0.01
M_TOTAL = float(B * N)         # BN statistic count (full batch)

KC = C // 128                  # 4 channel chunks of x
KM = CMID // 128               # 2 channel chunks of mid features


def build_kernel():
    nc = bacc.Bacc("TRN2", target_bir_lowering=False, debug=False,
                   num_devices=NCORES)

    # ---- DRAM I/O -------------------------------------------------------
    x1d = nc.dram_tensor("x1s", [128, 4096], F32, kind="ExternalInput")
    x2d = nc.dram_tensor("x2s", [128, 4096], F32, kind="ExternalInput")
    xd = nc.dram_tensor("xs", [128, 4096], BF16, kind="ExternalInput")
    w1d = nc.dram_tensor("w1t", [128, 1024], F32, kind="ExternalInput")
    w2d = nc.dram_tensor("w2t", [128, 9216], F32, kind="ExternalInput")
    # vecs rows: 0=b2, 1=gamma, 2=bn_bias, 3=beta(col0), 4=b1(cols 0:256)
    vecd = nc.dram_tensor("vecs", [8, 512], F32, kind="ExternalInput")
    outd = nc.dram_tensor("out", [128, 4096], F32, kind="ExternalOutput")

    cc_in = nc.dram_tensor("cc_in", [1, 2048], F32, kind="Internal")
    cc_out = nc.dram_tensor("cc_out", [1, 2048], F32, kind="Internal",
                            addr_space="Shared")
    cw_in = nc.dram_tensor("cw_in", [1, 8], F32, kind="Internal")
    cw_out = nc.dram_tensor("cw_out", [1, 8], F32, kind="Internal",
                            addr_space="Shared")
    sinkd = nc.dram_tensor("sinkd", [128, 512], F32, kind="Internal")

    with TileContext(nc, num_cores=NCORES) as tc:
        with (
            tc.tile_pool(name="const", bufs=1) as const,
            tc.tile_pool(name="big", bufs=1) as big,
            tc.tile_pool(name="work", bufs=2) as work,
            tc.tile_pool(name="vec", bufs=1) as vec,
            tc.tile_pool(name="ps", bufs=5, space="PSUM") as ps,
            tc.tile_pool(name="psstat", bufs=2, space="PSUM") as psstat,
        ):
            # ---- tiny constants ----------------------------------------
            identb = const.tile([128, 128], BF16)
            make_identity(nc, identb)
            ones_col_f = const.tile([128, 1], F32)
            nc.vector.memset(ones_col_f[:], 1.0)
            ones_col = ones_col_f[:].bitcast(F32R)
            nc.scalar.copy(ones_col, ones_col_f[:])
            ones_row_f = const.tile([1, 128], F32)
            nc.vector.memset(ones_row_f[:], 1.0)
            ones_row = ones_row_f[:].bitcast(F32R)
            nc.scalar.copy(ones_row, ones_row_f[:])
            # scratch for PE warm-up matmuls
            scratch_f = const.tile([128, 512], F32)
            nc.vector.memset(scratch_f[:], 0.5)
            scratch = scratch_f[:].bitcast(F32R)
            nc.scalar.copy(scratch, scratch_f[:])
            # act-table prime: force the single covering set (ln+exp) early
            prime = const.tile([1, 8], F32)
            nc.vector.memset(prime[:], 1.0)
            nc.scalar.activation(prime[:], prime[:], AF.Ln)
            nc.scalar.activation(prime[:], prime[:], AF.Exp)
            nc.scalar.activation(prime[:], prime[:], AF.Prelu,
                                 alpha=LRELU_SLOPE)
            # const APs for ACT scale/bias on the rsqrt path
            epsc = const.tile([128, 1], F32)
            nc.vector.memset(epsc[:], BN_EPS)
            invm = const.tile([128, 1], F32)
            nc.vector.memset(invm[:], 1.0 / M_TOTAL)
            mhalf = const.tile([128, 1], F32)
            nc.vector.memset(mhalf[:], -0.5)

            # ---- PE warm-up: get HAM to 2.4 GHz while input DMAs stream
            warm_sink = const.tile([128, 512], F32)
            for i in range(12):
                wacc = ps.tile([128, 512], F32, tag="warm", bufs=1)
                nc.tensor.matmul(wacc[:], scratch[:, 0:128], scratch,
                                 start=True, stop=True)
                if i == 11:
                    nc.scalar.copy(warm_sink[:], wacc[:])
            nc.sync.dma_start(out=sinkd[:], in_=warm_sink[:])

            # ---- small vector constants --------------------------------
            b2row = const.tile([1, 512], F32R)
            nc.sync.dma_start(out=b2row[:], in_=vecd[0:1, :].bitcast(F32R))
            b1pp = const.tile([128, KM], F32)
            for m in range(KM):
                nc.sync.dma_start(out=b1pp[:, m:m + 1],
                                  in_=vecd[4:5, 128 * m:128 * (m + 1)])
            # gb: gamma | bnb (used by both block chains)
            gb = const.tile([1, 1024], F32R)
            nc.sync.dma_start(out=gb[0:1, 0:512], in_=vecd[1:2, :].bitcast(F32R))
            nc.sync.dma_start(out=gb[0:1, 512:1024], in_=vecd[2:3, :].bitcast(F32R))
            gpp = const.tile([128, KC], F32)
            nc.sync.dma_start(
                out=gpp[:],
                in_=vecd[1:2, :].rearrange("o (j p) -> o p j", p=128))
            betar = const.tile([1, 1], F32)
            nc.sync.dma_start(out=betar[:], in_=vecd[3:4, 0:1])
            # beta broadcast per partition (AR-independent, done at start)
            bps = ps.tile([128, 1], F32, tag="warm", bufs=1)
            nc.tensor.matmul(bps[:], ones_row_f[:], betar[:],
                             start=True, stop=True)
            betapp = const.tile([128, 1], F32)
            nc.scalar.copy(betapp[:], bps[:])

            # ---- warmup AllReduce (pays collective setup; also used as a
            # cross-core alignment gate mid block 1) ----------------------
            warm = const.tile([1, 8], F32)
            nc.vector.memset(warm[:], 1.0)
            nc.sync.dma_start(out=cw_in[:], in_=warm[:])
            nc.gpsimd.collective_compute(
                "AllReduce", ALU.add,
                replica_groups=[list(range(NCORES))],
                ins=[cw_in[:]], outs=[cw_out[:]])
            warmres = const.tile([1, 8], F32)
            align_dma = nc.sync.dma_start(out=warmres[:], in_=cw_out[:])

            # ---- big input DMAs (order = consumption order) ------------
            w1t = big.tile([128, 1024], F32R)
            nc.sync.dma_start(out=w1t[:], in_=w1d[:].bitcast(F32R))
            x1s = work.tile([128, 4096], F32R, tag="xin")
            for j in range(4):
                nc.sync.dma_start(out=x1s[:, 1024 * j:1024 * (j + 1)],
                                  in_=x1d[:, 1024 * j:1024 * (j + 1)].bitcast(F32R))
            w2t = big.tile([128, 9216], F32R)
            for j in range(6):
                nc.sync.dma_start(out=w2t[:, 1536 * j:1536 * (j + 1)],
                                  in_=w2d[:, 1536 * j:1536 * (j + 1)].bitcast(F32R))
            x2s = work.tile([128, 4096], F32R, tag="xin")
            for j in range(4):
                nc.sync.dma_start(out=x2s[:, 1024 * j:1024 * (j + 1)],
                                  in_=x2d[:, 1024 * j:1024 * (j + 1)].bitcast(F32R))

            # ---- conv state --------------------------------------------
            y1pa_f = big.tile([128, 3 * KM * NROW], F32)
            nc.vector.memset(y1pa_f[:], 0.0)
            y1pa = y1pa_f[:].bitcast(F32R)
            nc.scalar.copy(y1pa, y1pa_f[:])
            y1pb = y1pa
            f1t = big.tile([128, 4096], F32R)
            f2t = big.tile([128, 4096], F32R)
            # local stats staging (r1 | s1 | r2 | s2), also feeds u/w
            statsb = vec.tile([1, 2048], F32R)

            def conv1(xin, y1p):
                accs = [[ps.tile([128, 512], F32, tag="ps", name="c1acc")
                         for _ in range(2)] for _ in range(KM)]
                for k in range(KC):
                    for m in range(KM):
                        for n2 in range(2):
                            nc.tensor.matmul(
                                accs[m][n2][:],
                                w1t[:, 256 * k + 128 * m:256 * k + 128 * (m + 1)],
                                xin[:, 1024 * k + 512 * n2:1024 * k + 512 * (n2 + 1)],
                                start=(k == 0), stop=(k == KC - 1))
                def y1base(kw, k):
                    return (kw * KM + k) * NROW
                for m in range(KM):
                    for n2 in range(2):
                        acc = accs[m][n2]
                        accv = acc[:].rearrange("p (r c) -> p r c", c=W)
                        row0 = (1 + 16 * n2) * W
                        # center copy (kw=1): straight contiguous store
                        nc.scalar.activation(
                            y1p[:, y1base(1, m) + row0:y1base(1, m) + row0 + 512],
                            acc[:], AF.Identity, bias=b1pp[:, m:m + 1])
                        # kw=0: shift right one col (src cols 0..30 -> 1..31)
                        d0 = y1p[:, y1base(0, m):y1base(0, m) + NROW].rearrange(
                            "p (r c) -> p r c", c=W)
                        nc.scalar.activation(
                            d0[:, 1 + 16 * n2:17 + 16 * n2, 1:32],
                            accv[:, :, 0:31], AF.Identity,
                            bias=b1pp[:, m:m + 1])
                        # kw=2: shift left one col (src cols 1..31 -> 0..30)
                        d2 = y1p[:, y1base(2, m):y1base(2, m) + NROW].rearrange(
                            "p (r c) -> p r c", c=W)
                        nc.scalar.activation(
                            d2[:, 1 + 16 * n2:17 + 16 * n2, 0:31],
                            accv[:, :, 1:32], AF.Identity,
                            bias=b1pp[:, m:m + 1])

            def win(y1p, s, kh, kw, k):
                off = (kw * KM + k) * NROW + (4 * s + kh) * W
                return y1p[:, off:off + 128]

            def evac_stats(accs, ft, racc, qacc, s_list, nchunks=8):
                # lrelu evac + per-channel (sum, sumsq) matmul accumulation
                for s in s_list:
                    nc.scalar.activation(ft[:, 512 * s:512 * (s + 1)],
                                         accs[s][:], AF.Prelu,
                                         alpha=LRELU_SLOPE)
                    sq = work.tile([128, 512], F32R, tag="sq")
                    nc.vector.tensor_mul(sq[:], ft[:, 512 * s:512 * (s + 1)],
                                         ft[:, 512 * s:512 * (s + 1)])
                    nc.tensor.matmul(racc[:], ones_col,
                                     ft[:, 512 * s:512 * (s + 1)],
                                     start=(s == 0), stop=(s == nchunks - 1))
                    nc.tensor.matmul(qacc[:], ones_col, sq[:],
                                     start=(s == 0), stop=(s == nchunks - 1))

            def stats_out(racc, qacc, si):
                # stage local stats in SBUF, then DMA to the collective
                nc.vector.tensor_copy(statsb[0:1, 1024 * si:1024 * si + 512],
                                      racc[:])
                nc.vector.tensor_copy(statsb[0:1, 1024 * si + 512:1024 * (si + 1)],
                                      qacc[:])
                nc.sync.dma_start(out=cc_in[0:1, 1024 * si:1024 * (si + 1)],
                                  in_=statsb[0:1, 1024 * si:1024 * (si + 1)].bitcast(F32))
                nc.gpsimd.collective_compute(
                    "AllReduce", ALU.add,
                    replica_groups=[list(range(NCORES))],
                    ins=[cc_in[0:1, 1024 * si:1024 * (si + 1)]],
                    outs=[cc_out[0:1, 1024 * si:1024 * (si + 1)]])

            def conv2_b1():
                # two tap-outer passes of 4 spatial chunks each: rides the
                # w2t DMA stream with at most 4+2 psum tiles live
                racc = psstat.tile([1, 512], F32, tag="st")
                qacc = psstat.tile([1, 512], F32, tag="st")
                gate_done = [False]
                for half in range(2):
                    accs = {}
                    for s in range(4 * half, 4 * half + 4):
                        acc = ps.tile([128, 512], F32, tag="ps")
                        accs[s] = acc
                        nc.tensor.matmul(acc[:], ones_row, b2row[:],
                                         start=True, stop=False)
                    for kh in range(3):
                        for kw in range(3):
                            t = kh * 3 + kw
                            for k in range(KM):
                                rhs = w2t[:, (2 * t + k) * 512:(2 * t + k + 1) * 512]
                                last = (kh == 2 and kw == 2 and k == KM - 1)
                                for s in range(4 * half, 4 * half + 4):
                                    mm = nc.tensor.matmul(
                                        accs[s][:], win(y1pa, s, kh, kw, k),
                                        rhs, start=False, stop=last)
                                if half == 1 and t == 3 and not gate_done[0]:
                                    # cross-core alignment: wait on warmup-AR
                                    # result once, mid-block, off the DMA path
                                    add_dep_helper(mm.ins, align_dma.ins,
                                                   sync=True,
                                                   reason="core-align gate")
                                    gate_done[0] = True
                        if half == 1 and kh == 1 and kw == 2:
                            # stats for the first half overlap these taps
                            evac_stats(accs1_saved, f1t, racc, qacc,
                                       range(0, 4))
                    if half == 0:
                        accs1_saved = accs
                evac_stats(accs, f1t, racc, qacc, range(4, 8))
                stats_out(racc, qacc, 0)

            def conv2_b2():
                racc = psstat.tile([1, 512], F32, tag="st")
                qacc = psstat.tile([1, 512], F32, tag="st")
                prev = None
                for s in range(8):
                    acc = ps.tile([128, 512], F32, tag="ps")
                    nc.tensor.matmul(acc[:], ones_row, b2row[:],
                                     start=True, stop=False)
                    for kh in range(3):
                        for kw in range(3):
                            t = kh * 3 + kw
                            for k in range(KM):
                                rhs = w2t[:, (2 * t + k) * 512:(2 * t + k + 1) * 512]
                                last = (kh == 2 and kw == 2 and k == KM - 1)
                                nc.tensor.matmul(acc[:],
                                                 win(y1pb, s, kh, kw, k),
                                                 rhs, start=False, stop=last)
                    if prev is not None:
                        evac_stats({prev: prev_acc}, f2t, racc, qacc, [prev])
                    prev, prev_acc = s, acc
                evac_stats({prev: prev_acc}, f2t, racc, qacc, [prev])
                stats_out(racc, qacc, 1)

            # rsqrt via single-table-set ln/exp: out = (x*sc+eps)^-0.5
            def rsqrt_chain(out_ap, in_ap, tmp_ap, scale_ap, bias_ap, mh_ap):
                nc.scalar.activation(tmp_ap, in_ap, AF.Ln,
                                     bias=bias_ap, scale=scale_ap)
                nc.scalar.activation(out_ap, tmp_ap, AF.Exp, scale=mh_ap)

            # per-block BN-affine chain in free layout [1,512]:
            # a_i = gamma*rsqrt(var+eps), b_i = bnb - mean*a_i
            def bn_chain(si, a_vec, b_vec, t1, t2):
                r = ar[0:1, 1024 * si:1024 * si + 512]
                s = ar[0:1, 1024 * si + 512:1024 * (si + 1)]
                gam = gb[0:1, 0:512]
                bnb = gb[0:1, 512:1024]
                nc.vector.tensor_mul(t1, r, r)                       # r^2
                nc.vector.scalar_tensor_tensor(                      # M*var
                    t2, t1, -1.0 / M_TOTAL, s, op0=ALU.mult, op1=ALU.add)
                rsqrt_chain(t1, t2, t1, invm[0:1, :], epsc[0:1, :],
                            mhalf[0:1, :])                           # invstd
                nc.vector.tensor_mul(a_vec, t1, gam)                 # a
                nc.vector.tensor_mul(t2, r, a_vec)                   # r*a
                nc.vector.scalar_tensor_tensor(                      # b
                    b_vec, t2, -1.0 / M_TOTAL, bnb, op0=ALU.mult, op1=ALU.add)

            def tail():
                # ---- block-2 BN chain (the only post-AR2 serial work) --
                nc.sync.dma_start(out=ar[0:1, 1024:2048],
                                  in_=cc_out[0:1, 1024:2048].bitcast(F32R))
                t1 = vec.tile([1, 512], F32R)
                t2 = vec.tile([1, 512], F32R)
                bn_chain(1, a2v[:], b2v[:], t1[:], t2[:])
                # w = a2*r2_loc + N*b2bn
                nc.vector.tensor_mul(wv[:], a2v[:], statsb[0:1, 1024:1536])
                nc.vector.scalar_tensor_tensor(
                    wv[:], b2v[:], float(N), wv[:], op0=ALU.mult, op1=ALU.add)
                # a2 broadcast tile [128, 512]
                bc = ps.tile([128, 512], F32, tag="ps")
                nc.tensor.matmul(bc[:], ones_row, a2v[:],
                                 start=True, stop=True)
                a2b = vec.tile([128, 512], F32)
                nc.scalar.copy(a2b[:], bc[:])

                scvec = vec.tile([128, KC], F32)
                for m in range(KC):
                    # rank-1 corrections, lazily per chunk (2 psum tiles max)
                    r1acc = ps.tile([128, 512], F32, tag="ps")
                    nc.tensor.matmul(r1acc[:], uv[0:1, 128 * m:128 * (m + 1)],
                                     b2v[:], start=True, stop=False)
                    nc.tensor.matmul(r1acc[:], b1v[0:1, 128 * m:128 * (m + 1)],
                                     wv[:], start=False, stop=True)
                    # scores = (S * a2[d]) * a1[c] + rank1
                    tmul = work.tile([128, 512], F32, tag="tmul")
                    nc.vector.tensor_mul(tmul[:], ssb[:, 512 * m:512 * (m + 1)],
                                         a2b[:])
                    sc = work.tile([128, 512], F32, tag="scores")
                    nc.vector.scalar_tensor_tensor(
                        sc[:], tmul[:], a1pp[:, m:m + 1], r1acc[:],
                        op0=ALU.mult, op1=ALU.add)
                    # E = exp(scores - rowmax) in bf16, sumexp for free
                    nmx = vec.tile([128, 1], F32, tag="nmx")
                    nc.vector.tensor_reduce(nmx[:], sc[:],
                                            axis=mybir.AxisListType.X,
                                            op=ALU.max, negate=True)
                    esum = vec.tile([128, 1], F32, tag="esum")
                    ee = work.tile([128, 512], BF16, tag="ee")
                    nc.scalar.activation(ee[:], sc[:], AF.Exp, bias=nmx[:],
                                         accum_out=esum[:])
                    nc.vector.reciprocal(esum[:], esum[:])
                    nc.vector.tensor_mul(scvec[:, m:m + 1], esum[:], betapp[:])
                    # transpose E chunk (bf16) into one psum bank
                    tpb = ps.tile([128, 512], BF16, tag="ps")
                    for j in range(KC):
                        nc.tensor.transpose(tpb[:, 128 * j:128 * (j + 1)],
                                            ee[:, 128 * j:128 * (j + 1)],
                                            identb[:])
                    etm = work.tile([128, 512], BF16, tag="etm")
                    nc.vector.tensor_copy(etm[:], tpb[:])
                    # out[c,n] = (beta/sumexp)[c] * sum_d E^T[d,c] x[d,n]
                    for n2 in range(2):
                        oacc = ps.tile([128, 512], F32, tag="ps")
                        for k in range(KC):
                            nc.tensor.matmul(
                                oacc[:], etm[:, 128 * k:128 * (k + 1)],
                                xs_ref[0][:, 1024 * k + 512 * n2:1024 * k + 512 * (n2 + 1)],
                                start=(k == 0), stop=(k == KC - 1))
                        ot = work.tile([128, 512], F32, tag="ot")
                        nc.scalar.mul(ot[:], oacc[:], scvec[:, m:m + 1])
                        nc.gpsimd.dma_start(
                            out=outd[:, 1024 * m + 512 * n2:1024 * m + 512 * (n2 + 1)],
                            in_=ot[:])

            # tiles shared across main/tail
            ar = vec.tile([1, 2048], F32R)
            a1v = vec.tile([1, 512], F32R)
            b1v = vec.tile([1, 512], F32R)
            a2v = vec.tile([1, 512], F32R)
            b2v = vec.tile([1, 512], F32R)
            uv = vec.tile([1, 512], F32R)
            wv = vec.tile([1, 512], F32R)
            a1pp = vec.tile([128, KC], F32)
            ssb = big.tile([128, 2048], F32)
            xs_ref = [None]

            def main_wrapper():
                conv1(x1s, y1pa)
                conv2_b1()
                xs = work.tile([128, 4096], BF16, tag="xin")
                xs_ref[0] = xs
                for j in range(2):
                    nc.sync.dma_start(out=xs[:, 2048 * j:2048 * (j + 1)],
                                      in_=xd[:, 2048 * j:2048 * (j + 1)])
                conv1(x2s, y1pb)
                nc.sync.dma_start(out=ar[0:1, 0:1024],
                                  in_=cc_out[0:1, 0:1024].bitcast(F32R))
                # block-1 BN chains (free layout + per-partition layout),
                # Ln's batched before Exp's: one table load each, hidden
                # under conv2(b2)'s compute
                t1 = vec.tile([1, 512], F32R)
                t2 = vec.tile([1, 512], F32R)
                r1 = ar[0:1, 0:512]
                s1 = ar[0:1, 512:1024]
                nc.vector.tensor_mul(t1[:], r1, r1)
                nc.vector.scalar_tensor_tensor(
                    t2[:], t1[:], -1.0 / M_TOTAL, s1,
                    op0=ALU.mult, op1=ALU.add)
                r1pp = vec.tile([128, KC], F32)
                s1pp = vec.tile([128, KC], F32)
                nc.sync.dma_start(
                    out=r1pp[:],
                    in_=cc_out[0:1, 0:512].rearrange("o (j p) -> o p j", p=128))
                nc.sync.dma_start(
                    out=s1pp[:],
                    in_=cc_out[0:1, 512:1024].rearrange("o (j p) -> o p j", p=128))
                p1 = vec.tile([128, KC], F32)
                nc.vector.tensor_mul(p1[:], r1pp[:], r1pp[:])
                nc.vector.scalar_tensor_tensor(
                    p1[:], p1[:], -1.0 / M_TOTAL, s1pp[:],
                    op0=ALU.mult, op1=ALU.add)
                nc.scalar.activation(t1[:], t2[:], AF.Ln,
                                     bias=epsc[0:1, :], scale=invm[0:1, :])
                nc.scalar.activation(p1[:], p1[:], AF.Ln,
                                     bias=epsc[:], scale=invm[:])
                nc.scalar.activation(t1[:], t1[:], AF.Exp, scale=mhalf[0:1, :])
                nc.scalar.activation(p1[:], p1[:], AF.Exp, scale=mhalf[:])
                nc.vector.tensor_mul(a1v[:], t1[:], gb[0:1, 0:512])
                nc.vector.tensor_mul(t2[:], r1, a1v[:])
                nc.vector.scalar_tensor_tensor(
                    b1v[:], t2[:], -1.0 / M_TOTAL, gb[0:1, 512:1024],
                    op0=ALU.mult, op1=ALU.add)
                nc.vector.tensor_mul(uv[:], a1v[:], statsb[0:1, 0:512])
                nc.vector.tensor_mul(a1pp[:], p1[:], gpp[:])
                conv2_b2()
                # pre-load the ln table set while the AR2 window is open, so
                # the tail's Ln needs no table switch
                nc.scalar.activation(prime[:], prime[:], AF.Ln)
                for m in range(KC):
                    sacc = ps.tile([128, 512], F32, tag="ps")
                    for k in range(8):
                        nc.tensor.matmul(
                            sacc[:],
                            f1t[:, 512 * k + 128 * m:512 * k + 128 * (m + 1)],
                            f2t[:, 512 * k:512 * (k + 1)],
                            start=(k == 0), stop=(k == 7))
                    nc.vector.tensor_copy(ssb[:, 512 * m:512 * (m + 1)],
                                          sacc[:].bitcast(F32))
                for i in range(14):
                    wacc = ps.tile([128, 512], F32, tag="warm", bufs=1)
                    nc.tensor.matmul(wacc[:], scratch[:, 0:128], scratch,
                                     start=True, stop=True)
                    if i == 13:
                        nc.scalar.copy(warm_sink[:], wacc[:])
                nc.sync.dma_start(out=sinkd[:], in_=warm_sink[:])
                tc.no_sync_barrier()
                tail()

            main_wrapper()

    nc.compile()
    return nc


_NC_CACHE = []


def _get_nc():
    if not _NC_CACHE:
        _NC_CACHE.append(build_kernel())
    return _NC_CACHE[0]


def _prep_shared(w1, b1, w2, b2, gamma, bn_bias, beta):
    w1m = w1.reshape(CMID, C).astype(np.float32)
    w1t = np.ascontiguousarray(
        w1m.T.reshape(KC, 128, CMID).transpose(1, 0, 2).reshape(128, KC * CMID))
    w2t = np.empty((128, 9216), dtype=np.float32)
    for kh in range(3):
        for kw in range(3):
            t = kh * 3 + kw
            wt = w2[:, :, kh, kw].T  # [256 in, 512 out]
            for k in range(KM):
                w2t[:, (2 * t + k) * 512:(2 * t + k + 1) * 512] = \
                    wt[128 * k:128 * (k + 1), :]
    vecs = np.zeros((8, 512), dtype=np.float32)
    vecs[0] = b2
    vecs[1] = gamma
    vecs[2] = bn_bias
    vecs[3, 0] = np.asarray(beta).reshape(-1)[0]
    vecs[4, :CMID] = b1
    return w1t, w2t, vecs


def _chunk_img(img):
    # [512, 1024] -> [128, 4096] with channel chunk k at cols 1024k
    return np.ascontiguousarray(
        img.reshape(KC, 128, N).transpose(1, 0, 2).reshape(128, KC * N))


def kernel(x, x1, x2, w1, b1, w2, b2, gamma, bn_bias, beta, **run_kw):
    import ml_dtypes
    nc = _get_nc()
    w1t, w2t, vecs = _prep_shared(w1, b1, w2, b2, gamma, bn_bias, beta)
    in_maps = []
    for i in range(NCORES):
        in_maps.append({
            "x1s": _chunk_img(np.asarray(x1[i], np.float32).reshape(C, N)),
            "x2s": _chunk_img(np.asarray(x2[i], np.float32).reshape(C, N)),
            "xs": _chunk_img(np.asarray(x[i], np.float32).reshape(C, N)).astype(ml_dtypes.bfloat16),
            "w1t": w1t, "w2t": w2t, "vecs": vecs,
        })
    res = run_bass_kernel_spmd(nc, in_maps, list(range(NCORES)), **run_kw)
    out = np.empty((B, C, H, W), dtype=np.float32)
    for i in range(NCORES):
        o = res.results[i]["out"]  # [128, 4096]
        out[i] = o.reshape(128, KC, N).transpose(1, 0, 2).reshape(C, H, W)
    if run_kw:
        kernel.last_results = res
    return out


# revision 19
# speedup vs baseline: 1.1206x; 1.1109x over previous
"""Trainium2 Bass kernel for nn_DisCA (dual conv-block + channel attention).

Data-parallel over batch: 8 batch items -> 8 NeuronCores, one image per core.
Conv weights / BN affine replicated. BatchNorm batch statistics via per-block
cross-core AllReduce of per-channel (sum, sumsq); BN affine folded into the
attention-score matrix so the raw Gram matmul overlaps the AllReduce:
    scores = a1[c]*a2[d]*S[c,d] + (a1*r1)[c]*b2bn[d] + b1bn[c]*(a2*r2 + N*b2bn)[d]

Key layout choices (per core):
  x/x1/x2: [128, 4096]  - channel-chunk k (4 chunks of 128ch) at cols 1024k,
                          spatial n=H*W=1024 flattened row-major (x in bf16).
  w1t:     [128, 1024]  - W1^T as 4 K-chunks of [128,256].
  w2t:     [128, 9216]  - per tap t, K-chunk k: W2[:,:,kh,kw]^T chunk at
                          cols (2t+k)*512.
  conv1 out Y1 is stored zero-padded 34x34 per channel chunk, so conv2's
  stationary operand for (s,kh,kw,k) is a strided [128,4,32] window and the
  3x3 conv is 9 shifted 1x1 matmuls with no edge fixups and a single store
  per conv1 psum.
  conv2 output F is produced TRANSPOSED ([n, c]) so score matmuls need no
  explicit transposes; softmax E is bf16, transposed via PE, and the final
  attention@x runs in bf16.
All activation functions used (identity/copy/square/parametric_relu/ln/exp)
live in ONE act table set, so no table switches on the critical path; the
BN rsqrt is computed as exp(-0.5*ln(var+eps)).
"""

import os
import sys

for _p in ("/opt/trn_rl_repo", "/root/.axon_site/_ro/trn_rl_repo"):
    if os.path.isdir(_p) and _p not in sys.path:
        sys.path.insert(0, _p)

import numpy as np

import concourse.bacc as bacc
import concourse.mybir as mybir
from concourse.tile import TileContext, add_dep_helper
from concourse.bass_utils import run_bass_kernel_spmd
from concourse.masks import make_identity

F32 = mybir.dt.float32
F32R = mybir.dt.float32r
BF16 = mybir.dt.bfloat16
AF = mybir.ActivationFunctionType
ALU = mybir.AluOpType

NCORES = 8
B, C, H, W = 8, 512, 32, 32
N = H * W                      # 1024 spatial positions per image
CMID = 256                     # conv1 output channels
HP = H + 2                     # padded rows
NROW = HP * W                  # 1088 elems per shifted copy per chunk
BN_EPS = 1e-5
LRELU_SLOPE = 0.01
M_TOTAL = float(B * N)         # BN statistic count (full batch)

KC = C // 128                  # 4 channel chunks of x
KM = CMID // 128               # 2 channel chunks of mid features


def build_kernel():
    nc = bacc.Bacc("TRN2", target_bir_lowering=False, debug=False,
                   num_devices=NCORES)

    # ---- DRAM I/O -------------------------------------------------------
    x1d = nc.dram_tensor("x1s", [128, 4096], F32, kind="ExternalInput")
    x2d = nc.dram_tensor("x2s", [128, 4096], F32, kind="ExternalInput")
    xd = nc.dram_tensor("xs", [128, 4096], BF16, kind="ExternalInput")
    w1d = nc.dram_tensor("w1t", [128, 1024], F32, kind="ExternalInput")
    w2d = nc.dram_tensor("w2t", [128, 9216], F32, kind="ExternalInput")
    # vecs rows: 0=b2, 1=gamma, 2=bn_bias, 3=beta(col0), 4=b1(cols 0:256)
    vecd = nc.dram_tensor("vecs", [8, 512], F32, kind="ExternalInput")
    outd = nc.dram_tensor("out", [128, 4096], F32, kind="ExternalOutput")

    cc_in = nc.dram_tensor("cc_in", [1, 2048], F32, kind="Internal")
    cc_out = nc.dram_tensor("cc_out", [1, 2048], F32, kind="Internal",
                            addr_space="Shared")
    cw_in = nc.dram_tensor("cw_in", [1, 8], F32, kind="Internal")
    cw_out = nc.dram_tensor("cw_out", [1, 8], F32, kind="Internal",
                            addr_space="Shared")
    sinkd = nc.dram_tensor("sinkd", [128, 512], F32, kind="Internal")

    with TileContext(nc, num_cores=NCORES) as tc:
        with (
            tc.tile_pool(name="const", bufs=1) as const,
            tc.tile_pool(name="big", bufs=1) as big,
            tc.tile_pool(name="work", bufs=2) as work,
            tc.tile_pool(name="vec", bufs=1) as vec,
            tc.tile_pool(name="ps", bufs=4, space="PSUM") as ps,
            tc.tile_pool(name="psstat", bufs=2, space="PSUM") as psstat,
        ):
            # ---- tiny constants ----------------------------------------
            identb = const.tile([128, 128], BF16)
            make_identity(nc, identb)
            ones_col_f = const.tile([128, 1], F32)
            nc.vector.memset(ones_col_f[:], 1.0)
            ones_col = ones_col_f[:].bitcast(F32R)
            nc.scalar.copy(ones_col, ones_col_f[:])
            ones_row_f = const.tile([1, 128], F32)
            nc.vector.memset(ones_row_f[:], 1.0)
            ones_row = ones_row_f[:].bitcast(F32R)
            nc.scalar.copy(ones_row, ones_row_f[:])
            # scratch for PE warm-up matmuls
            scratch_f = const.tile([128, 512], F32)
            nc.vector.memset(scratch_f[:], 0.5)
            scratch = scratch_f[:].bitcast(F32R)
            nc.scalar.copy(scratch, scratch_f[:])
            # act-table prime: force the single covering set (ln+exp) early
            prime = const.tile([1, 8], F32)
            nc.vector.memset(prime[:], 1.0)
            nc.scalar.activation(prime[:], prime[:], AF.Ln)
            nc.scalar.activation(prime[:], prime[:], AF.Exp)
            nc.scalar.activation(prime[:], prime[:], AF.Prelu,
                                 alpha=LRELU_SLOPE)
            # const APs for ACT scale/bias on the rsqrt path
            epsc = const.tile([128, 1], F32)
            nc.vector.memset(epsc[:], BN_EPS)
            invm = const.tile([128, 1], F32)
            nc.vector.memset(invm[:], 1.0 / M_TOTAL)
            mhalf = const.tile([128, 1], F32)
            nc.vector.memset(mhalf[:], -0.5)

            # ---- PE warm-up: get HAM to 2.4 GHz while input DMAs stream
            warm_sink = const.tile([128, 512], F32)
            for i in range(12):
                wacc = psstat.tile([128, 512], F32, tag="r1", name="wacc")
                nc.tensor.matmul(wacc[:], scratch[:, 0:128], scratch,
                                 start=True, stop=True)
                if i == 11:
                    nc.scalar.copy(warm_sink[:], wacc[:])
            nc.sync.dma_start(out=sinkd[:], in_=warm_sink[:])

            # ---- small vector constants --------------------------------
            b2row = const.tile([1, 512], F32R)
            nc.sync.dma_start(out=b2row[:], in_=vecd[0:1, :].bitcast(F32R))
            b1pp = const.tile([128, KM], F32)
            for m in range(KM):
                nc.sync.dma_start(out=b1pp[:, m:m + 1],
                                  in_=vecd[4:5, 128 * m:128 * (m + 1)])
            # gb: gamma | bnb (used by both block chains)
            gb = const.tile([1, 1024], F32)
            nc.sync.dma_start(out=gb[0:1, 0:512], in_=vecd[1:2, :])
            nc.sync.dma_start(out=gb[0:1, 512:1024], in_=vecd[2:3, :])
            gpp = const.tile([128, KC], F32)
            nc.sync.dma_start(
                out=gpp[:],
                in_=vecd[1:2, :].rearrange("o (j p) -> o p j", p=128))
            betar = const.tile([1, 1], F32)
            nc.sync.dma_start(out=betar[:], in_=vecd[3:4, 0:1])
            # beta broadcast per partition (AR-independent, done at start)
            bps = psstat.tile([128, 1], F32, tag="st", name="bps")
            nc.tensor.matmul(bps[:], ones_row_f[:], betar[:],
                             start=True, stop=True)
            betapp = const.tile([128, 1], F32)
            nc.scalar.copy(betapp[:], bps[:])

            # b2 broadcast [128, 512] for the conv2 bias add
            b2ps = psstat.tile([128, 512], F32, tag="r1", name="wacc")
            nc.tensor.matmul(b2ps[:], ones_row, b2row[:], start=True,
                             stop=True)
            b2b = const.tile([128, 512], F32)
            nc.scalar.copy(b2b[:], b2ps[:])

            # ---- warmup AllReduce (pays collective setup; also used as a
            # cross-core alignment gate mid block 1) ----------------------
            warm = const.tile([1, 8], F32)
            nc.vector.memset(warm[:], 1.0)
            nc.sync.dma_start(out=cw_in[:], in_=warm[:])
            nc.gpsimd.collective_compute(
                "AllReduce", ALU.add,
                replica_groups=[list(range(NCORES))],
                ins=[cw_in[:]], outs=[cw_out[:]])


            # ---- big input DMAs (order = consumption order) ------------
            w1t = big.tile([128, 1024], F32R)
            nc.sync.dma_start(out=w1t[:], in_=w1d[:].bitcast(F32R))
            x1s = work.tile([128, 4096], F32R, tag="xin")
            for j in range(4):
                nc.sync.dma_start(out=x1s[:, 1024 * j:1024 * (j + 1)],
                                  in_=x1d[:, 1024 * j:1024 * (j + 1)].bitcast(F32R))
            w2t = big.tile([128, 9216], F32R)
            for j in range(6):
                nc.sync.dma_start(out=w2t[:, 1536 * j:1536 * (j + 1)],
                                  in_=w2d[:, 1536 * j:1536 * (j + 1)].bitcast(F32R))
            x2s = work.tile([128, 4096], F32R, tag="xin")
            for j in range(4):
                nc.sync.dma_start(out=x2s[:, 1024 * j:1024 * (j + 1)],
                                  in_=x2d[:, 1024 * j:1024 * (j + 1)].bitcast(F32R))

            # ---- conv state --------------------------------------------
            y1pa_f = big.tile([128, 3 * KM * NROW], F32)
            nc.vector.memset(y1pa_f[:], 0.0)
            y1pa = y1pa_f[:].bitcast(F32R)
            nc.scalar.copy(y1pa, y1pa_f[:])
            y1pb = y1pa
            f1t = big.tile([128, 4096], F32R)
            f2t = big.tile([128, 4096], F32R)
            # local stats staging (r1 | s1 | r2 | s2), also feeds u/w
            statsb = vec.tile([1, 2048], F32)

            def conv1(xin, y1p):
                accs = [[ps.tile([128, 512], F32, tag="ps", name="c1acc")
                         for _ in range(2)] for _ in range(KM)]
                for k in range(KC):
                    for m in range(KM):
                        for n2 in range(2):
                            nc.tensor.matmul(
                                accs[m][n2][:],
                                w1t[:, 256 * k + 128 * m:256 * k + 128 * (m + 1)],
                                xin[:, 1024 * k + 512 * n2:1024 * k + 512 * (n2 + 1)],
                                start=(k == 0), stop=(k == KC - 1))
                def y1base(kw, k):
                    return (kw * KM + k) * NROW
                for m in range(KM):
                    for n2 in range(2):
                        acc = accs[m][n2]
                        accv = acc[:].rearrange("p (r c) -> p r c", c=W)
                        row0 = (1 + 16 * n2) * W
                        # center copy (kw=1): straight contiguous store
                        nc.scalar.activation(
                            y1p[:, y1base(1, m) + row0:y1base(1, m) + row0 + 512],
                            acc[:], AF.Identity, bias=b1pp[:, m:m + 1])
                        # kw=0: shift right one col (src cols 0..30 -> 1..31)
                        d0 = y1p[:, y1base(0, m):y1base(0, m) + NROW].rearrange(
                            "p (r c) -> p r c", c=W)
                        nc.scalar.activation(
                            d0[:, 1 + 16 * n2:17 + 16 * n2, 1:32],
                            accv[:, :, 0:31], AF.Identity,
                            bias=b1pp[:, m:m + 1])
                        # kw=2: shift left one col (src cols 1..31 -> 0..30)
                        d2 = y1p[:, y1base(2, m):y1base(2, m) + NROW].rearrange(
                            "p (r c) -> p r c", c=W)
                        nc.scalar.activation(
                            d2[:, 1 + 16 * n2:17 + 16 * n2, 0:31],
                            accv[:, :, 1:32], AF.Identity,
                            bias=b1pp[:, m:m + 1])

            def win(y1p, s, kh, kw, k):
                off = (kw * KM + k) * NROW + (4 * s + kh) * W
                return y1p[:, off:off + 128]

            def evac_stats(accs, ft, racc, qacc, s_list, nchunks=8):
                # lrelu evac + per-channel (sum, sumsq) matmul accumulation
                for s in s_list:
                    nc.vector.tensor_add(accs[s][:], accs[s][:], b2b[:])
                    nc.scalar.activation(ft[:, 512 * s:512 * (s + 1)],
                                         accs[s][:], AF.Prelu,
                                         alpha=LRELU_SLOPE)
                    sq = work.tile([128, 512], F32R, tag="sq")
                    nc.vector.tensor_mul(sq[:], ft[:, 512 * s:512 * (s + 1)],
                                         ft[:, 512 * s:512 * (s + 1)])
                    nc.tensor.matmul(racc[:], ones_col,
                                     ft[:, 512 * s:512 * (s + 1)],
                                     start=(s == 0), stop=(s == nchunks - 1))
                    nc.tensor.matmul(qacc[:], ones_col, sq[:],
                                     start=(s == 0), stop=(s == nchunks - 1))

            def stats_out(racc, qacc, si):
                # stage local stats in SBUF, then DMA to the collective
                nc.vector.tensor_copy(statsb[0:1, 1024 * si:1024 * si + 512],
                                      racc[:])
                nc.vector.tensor_copy(statsb[0:1, 1024 * si + 512:1024 * (si + 1)],
                                      qacc[:])
                nc.sync.dma_start(out=cc_in[0:1, 1024 * si:1024 * (si + 1)],
                                  in_=statsb[0:1, 1024 * si:1024 * (si + 1)])
                nc.gpsimd.collective_compute(
                    "AllReduce", ALU.add,
                    replica_groups=[list(range(NCORES))],
                    ins=[cc_in[0:1, 1024 * si:1024 * (si + 1)]],
                    outs=[cc_out[0:1, 1024 * si:1024 * (si + 1)]])

            def conv2_b1():
                # two tap-outer passes of 4 spatial chunks each: rides the
                # w2t DMA stream with at most 4+2 psum tiles live
                racc = psstat.tile([1, 512], F32, tag="st")
                qacc = psstat.tile([1, 512], F32, tag="st")
                for half in range(2):
                    accs = {}
                    for s in range(4 * half, 4 * half + 4):
                        accs[s] = ps.tile([128, 512], F32, tag="ps",
                                          name="c2acc")
                    for kh in range(3):
                        for kw in range(3):
                            t = kh * 3 + kw
                            for k in range(KM):
                                rhs = w2t[:, (2 * t + k) * 512:(2 * t + k + 1) * 512]
                                first = (kh == 0 and kw == 0 and k == 0)
                                last = (kh == 2 and kw == 2 and k == KM - 1)
                                for s in range(4 * half, 4 * half + 4):
                                    nc.tensor.matmul(
                                        accs[s][:], win(y1pa, s, kh, kw, k),
                                        rhs, start=first, stop=last)
                        if half == 1 and kh == 1 and kw == 2:
                            # stats for the first half overlap these taps
                            evac_stats(accs1_saved, f1t, racc, qacc,
                                       range(0, 4))
                    if half == 0:
                        accs1_saved = accs
                evac_stats(accs, f1t, racc, qacc, range(4, 8))
                stats_out(racc, qacc, 0)

            def conv2_b2():
                racc = psstat.tile([1, 512], F32, tag="st")
                qacc = psstat.tile([1, 512], F32, tag="st")
                prev = None
                for s in range(8):
                    acc = ps.tile([128, 512], F32, tag="ps")
                    for kh in range(3):
                        for kw in range(3):
                            t = kh * 3 + kw
                            for k in range(KM):
                                rhs = w2t[:, (2 * t + k) * 512:(2 * t + k + 1) * 512]
                                first = (kh == 0 and kw == 0 and k == 0)
                                last = (kh == 2 and kw == 2 and k == KM - 1)
                                nc.tensor.matmul(acc[:],
                                                 win(y1pb, s, kh, kw, k),
                                                 rhs, start=first, stop=last)
                    if prev is not None:
                        evac_stats({prev: prev_acc}, f2t, racc, qacc, [prev])
                    prev, prev_acc = s, acc
                evac_stats({prev: prev_acc}, f2t, racc, qacc, [prev])
                stats_out(racc, qacc, 1)

            # rsqrt via single-table-set ln/exp: out = (x*sc+eps)^-0.5
            def rsqrt_chain(out_ap, in_ap, tmp_ap, scale_ap, bias_ap, mh_ap):
                nc.scalar.activation(tmp_ap, in_ap, AF.Ln,
                                     bias=bias_ap, scale=scale_ap)
                nc.scalar.activation(out_ap, tmp_ap, AF.Exp, scale=mh_ap)

            # per-block BN-affine chain in free layout [1,512]:
            # a_i = gamma*rsqrt(var+eps), b_i = bnb - mean*a_i
            def bn_chain(si, a_vec, b_vec, t1, t2):
                r = ar[0:1, 1024 * si:1024 * si + 512]
                s = ar[0:1, 1024 * si + 512:1024 * (si + 1)]
                gam = gb[0:1, 0:512]
                bnb = gb[0:1, 512:1024]
                nc.vector.tensor_mul(t1, r, r)                       # r^2
                nc.vector.scalar_tensor_tensor(                      # M*var
                    t2, t1, -1.0 / M_TOTAL, s, op0=ALU.mult, op1=ALU.add)
                rsqrt_chain(t1, t2, t1, invm[0:1, :], epsc[0:1, :],
                            mhalf[0:1, :])                           # invstd
                nc.vector.tensor_mul(a_vec, t1, gam)                 # a
                nc.vector.tensor_mul(t2, r, a_vec)                   # r*a
                nc.vector.scalar_tensor_tensor(                      # b
                    b_vec, t2, -1.0 / M_TOTAL, bnb, op0=ALU.mult, op1=ALU.add)

            def tail():
                # ---- block-2 BN chain (the only post-AR2 serial work) --
                nc.sync.dma_start(out=ar[0:1, 1024:2048],
                                  in_=cc_out[0:1, 1024:2048])
                t1 = vec.tile([1, 512], F32)
                t2 = vec.tile([1, 512], F32)
                bn_chain(1, a2v[:], vpack[0:1, 1536:2048], t1[:], t2[:])
                # w = a2*r2_loc + N*b2bn
                nc.vector.tensor_mul(vpack[0:1, 1024:1536], a2v[:],
                                     statsb[0:1, 1024:1536])
                nc.vector.scalar_tensor_tensor(
                    vpack[0:1, 1024:1536], vpack[0:1, 1536:2048], float(N),
                    vpack[0:1, 1024:1536], op0=ALU.mult, op1=ALU.add)
                # single f32r rounding of [w | b2bn] for the rank-1 matmuls
                nc.vector.tensor_copy(r1vecs[0:1, 1024:2048],
                                      vpack[0:1, 1024:2048].bitcast(F32R))
                # a2 broadcast tile [128, 512] (full-precision fp32 matmul)
                bc = ps.tile([128, 512], F32, tag="ps")
                nc.tensor.matmul(bc[:], ones_row_f[:], a2v[:],
                                 start=True, stop=True)
                a2b = vec.tile([128, 512], F32)
                nc.scalar.copy(a2b[:], bc[:])

                r1accs = {}

                def rank1(m):
                    acc = psstat.tile([128, 512], F32, tag="r1",
                                      name="r1acc")
                    nc.tensor.matmul(acc[:],
                                     r1vecs[0:1, 128 * m:128 * (m + 1)],
                                     r1vecs[0:1, 1536:2048],
                                     start=True, stop=False)
                    nc.tensor.matmul(acc[:],
                                     r1vecs[0:1, 512 + 128 * m:512 + 128 * (m + 1)],
                                     r1vecs[0:1, 1024:1536],
                                     start=False, stop=True)
                    r1accs[m] = acc

                rank1(0)
                rank1(1)
                scvec = vec.tile([128, KC], F32)
                for m in range(KC):
                    # scores = (S * a2[d]) * a1[c] + rank1
                    tmul = work.tile([128, 512], F32, tag="tmul")
                    nc.vector.tensor_mul(tmul[:], ssb[:, 512 * m:512 * (m + 1)],
                                         a2b[:])
                    sc = work.tile([128, 512], F32, tag="scores")
                    nc.vector.scalar_tensor_tensor(
                        sc[:], tmul[:], a1pp[:, m:m + 1], r1accs[m][:],
                        op0=ALU.mult, op1=ALU.add)
                    # E = exp(scores - rowmax) in bf16, sumexp for free
                    nmx = vec.tile([128, 1], F32, tag="nmx")
                    nc.vector.tensor_reduce(nmx[:], sc[:],
                                            axis=mybir.AxisListType.X,
                                            op=ALU.max, negate=True)
                    esum = vec.tile([128, 1], F32, tag="esum")
                    ee = work.tile([128, 512], BF16, tag="ee")
                    nc.scalar.activation(ee[:], sc[:], AF.Exp, bias=nmx[:],
                                         accum_out=esum[:])
                    nc.vector.reciprocal(esum[:], esum[:])
                    nc.vector.tensor_mul(scvec[:, m:m + 1], esum[:], betapp[:])
                    if m + 2 < KC:
                        rank1(m + 2)
                    # transpose E chunk (bf16) into one psum bank
                    tpb = ps.tile([128, 512], BF16, tag="ps")
                    for j in range(KC):
                        nc.tensor.transpose(tpb[:, 128 * j:128 * (j + 1)],
                                            ee[:, 128 * j:128 * (j + 1)],
                                            identb[:])
                    etm = work.tile([128, 512], BF16, tag="etm")
                    nc.vector.tensor_copy(etm[:], tpb[:])
                    # out[c,n] = (beta/sumexp)[c] * sum_d E^T[d,c] x[d,n]
                    for n2 in range(2):
                        oacc = ps.tile([128, 512], F32, tag="ps")
                        for k in range(KC):
                            nc.tensor.matmul(
                                oacc[:], etm[:, 128 * k:128 * (k + 1)],
                                xs_ref[0][:, 1024 * k + 512 * n2:1024 * k + 512 * (n2 + 1)],
                                start=(k == 0), stop=(k == KC - 1))
                        ot = work.tile([128, 512], F32, tag="ot")
                        nc.scalar.mul(ot[:], oacc[:], scvec[:, m:m + 1])
                        nc.gpsimd.dma_start(
                            out=outd[:, 1024 * m + 512 * n2:1024 * m + 512 * (n2 + 1)],
                            in_=ot[:])

            # tiles shared across main/tail
            ar = vec.tile([1, 2048], F32)
            a1v = vec.tile([1, 512], F32)
            a2v = vec.tile([1, 512], F32)
            # packed correction vectors: u | b1bn | w | b2bn (fp32 chain
            # results; r1vecs is the once-rounded f32r copy for matmuls)
            vpack = vec.tile([1, 2048], F32)
            r1vecs = vec.tile([1, 2048], F32R)
            a1pp = vec.tile([128, KC], F32)
            ssb = big.tile([128, 2048], F32, tag="w2t")
            xs_ref = [None]

            def main_wrapper():
                conv1(x1s, y1pa)
                conv2_b1()
                xs = work.tile([128, 4096], BF16, tag="xin")
                xs_ref[0] = xs
                for j in range(2):
                    nc.sync.dma_start(out=xs[:, 2048 * j:2048 * (j + 1)],
                                      in_=xd[:, 2048 * j:2048 * (j + 1)])
                conv1(x2s, y1pb)
                nc.sync.dma_start(out=ar[0:1, 0:1024],
                                  in_=cc_out[0:1, 0:1024])
                # block-1 BN chains (free layout + per-partition layout),
                # Ln's batched before Exp's: one table load each, hidden
                # under conv2(b2)'s compute
                t1 = vec.tile([1, 512], F32)
                t2 = vec.tile([1, 512], F32)
                r1 = ar[0:1, 0:512]
                s1 = ar[0:1, 512:1024]
                nc.vector.tensor_mul(t1[:], r1, r1)
                nc.vector.scalar_tensor_tensor(
                    t2[:], t1[:], -1.0 / M_TOTAL, s1,
                    op0=ALU.mult, op1=ALU.add)
                r1pp = vec.tile([128, KC], F32)
                s1pp = vec.tile([128, KC], F32)
                nc.sync.dma_start(
                    out=r1pp[:],
                    in_=cc_out[0:1, 0:512].rearrange("o (j p) -> o p j", p=128))
                nc.sync.dma_start(
                    out=s1pp[:],
                    in_=cc_out[0:1, 512:1024].rearrange("o (j p) -> o p j", p=128))
                p1 = vec.tile([128, KC], F32)
                nc.vector.tensor_mul(p1[:], r1pp[:], r1pp[:])
                nc.vector.scalar_tensor_tensor(
                    p1[:], p1[:], -1.0 / M_TOTAL, s1pp[:],
                    op0=ALU.mult, op1=ALU.add)
                nc.scalar.activation(t1[:], t2[:], AF.Ln,
                                     bias=epsc[0:1, :], scale=invm[0:1, :])
                nc.scalar.activation(p1[:], p1[:], AF.Ln,
                                     bias=epsc[:], scale=invm[:])
                nc.scalar.activation(t1[:], t1[:], AF.Exp, scale=mhalf[0:1, :])
                nc.scalar.activation(p1[:], p1[:], AF.Exp, scale=mhalf[:])
                nc.vector.tensor_mul(a1v[:], t1[:], gb[0:1, 0:512])
                nc.vector.tensor_mul(t2[:], r1, a1v[:])
                nc.vector.scalar_tensor_tensor(
                    vpack[0:1, 512:1024], t2[:], -1.0 / M_TOTAL,
                    gb[0:1, 512:1024], op0=ALU.mult, op1=ALU.add)
                nc.vector.tensor_mul(vpack[0:1, 0:512], a1v[:],
                                     statsb[0:1, 0:512])
                nc.vector.tensor_copy(r1vecs[0:1, 0:1024],
                                      vpack[0:1, 0:1024].bitcast(F32R))
                nc.vector.tensor_mul(a1pp[:], p1[:], gpp[:])
                conv2_b2()
                # pre-load the ln table set while the AR2 window is open, so
                # the tail's Ln needs no table switch
                nc.scalar.activation(prime[:], prime[:], AF.Ln)
                for m in range(KC):
                    sacc = ps.tile([128, 512], F32, tag="ps")
                    for k in range(8):
                        nc.tensor.matmul(
                            sacc[:],
                            f1t[:, 512 * k + 128 * m:512 * k + 128 * (m + 1)],
                            f2t[:, 512 * k:512 * (k + 1)],
                            start=(k == 0), stop=(k == 7))
                    nc.vector.tensor_copy(ssb[:, 512 * m:512 * (m + 1)],
                                          sacc[:].bitcast(F32))
                for i in range(14):
                    wacc = psstat.tile([128, 512], F32, tag="r1", name="wacc")
                    nc.tensor.matmul(wacc[:], scratch[:, 0:128], scratch,
                                     start=True, stop=True)
                    if i == 13:
                        nc.scalar.copy(warm_sink[:], wacc[:])
                nc.sync.dma_start(out=sinkd[:], in_=warm_sink[:])
                tc.no_sync_barrier()
                tail()

            main_wrapper()

    nc.compile()
    return nc


_NC_CACHE = []


def _get_nc():
    if not _NC_CACHE:
        _NC_CACHE.append(build_kernel())
    return _NC_CACHE[0]


def _prep_shared(w1, b1, w2, b2, gamma, bn_bias, beta):
    w1m = w1.reshape(CMID, C).astype(np.float32)
    w1t = np.ascontiguousarray(
        w1m.T.reshape(KC, 128, CMID).transpose(1, 0, 2).reshape(128, KC * CMID))
    w2t = np.empty((128, 9216), dtype=np.float32)
    for kh in range(3):
        for kw in range(3):
            t = kh * 3 + kw
            wt = w2[:, :, kh, kw].T  # [256 in, 512 out]
            for k in range(KM):
                w2t[:, (2 * t + k) * 512:(2 * t + k + 1) * 512] = \
                    wt[128 * k:128 * (k + 1), :]
    vecs = np.zeros((8, 512), dtype=np.float32)
    vecs[0] = b2
    vecs[1] = gamma
    vecs[2] = bn_bias
    vecs[3, 0] = np.asarray(beta).reshape(-1)[0]
    vecs[4, :CMID] = b1
    return w1t, w2t, vecs


def _chunk_img(img):
    # [512, 1024] -> [128, 4096] with channel chunk k at cols 1024k
    return np.ascontiguousarray(
        img.reshape(KC, 128, N).transpose(1, 0, 2).reshape(128, KC * N))


def kernel(x, x1, x2, w1, b1, w2, b2, gamma, bn_bias, beta, **run_kw):
    import ml_dtypes
    nc = _get_nc()
    w1t, w2t, vecs = _prep_shared(w1, b1, w2, b2, gamma, bn_bias, beta)
    in_maps = []
    for i in range(NCORES):
        in_maps.append({
            "x1s": _chunk_img(np.asarray(x1[i], np.float32).reshape(C, N)),
            "x2s": _chunk_img(np.asarray(x2[i], np.float32).reshape(C, N)),
            "xs": _chunk_img(np.asarray(x[i], np.float32).reshape(C, N)).astype(ml_dtypes.bfloat16),
            "w1t": w1t, "w2t": w2t, "vecs": vecs,
        })
    res = run_bass_kernel_spmd(nc, in_maps, list(range(NCORES)), **run_kw)
    out = np.empty((B, C, H, W), dtype=np.float32)
    for i in range(NCORES):
        o = res.results[i]["out"]  # [128, 4096]
        out[i] = o.reshape(128, KC, N).transpose(1, 0, 2).reshape(C, H, W)
    if run_kw:
        kernel.last_results = res
    return out
